# revision 13
# baseline (speedup 1.0000x reference)
"""Trainium2 Bass kernel for nn_Attention_1580547974448.

Math insight: the reference uses raw .reshape (not a head-split transpose) on
[B,T,H*HD] -> [B,H,T,HD].  With B=4, T=4096, DIM=1024, H=16, HD=64 this makes
each "head" a contiguous 256-row slab of the flattened [B*T, DIM] = [16384,1024]
input: for slab s (rows 256s..256s+255),
    Q = (x_s @ Wq + bq)            viewed row-major as [4096, 64]
    S = Q^T K / sqrt(64)           [64, 64]
    P = softmax(S, axis=-1)
    O = P @ V^T                    [64, 4096], row-major == [256, 1024]
    y_s = O_v @ Wp + bp
i.e. the whole computation is block-diagonal over 64 independent slabs.
We shard 8 slabs (2048 rows) per NeuronCore -> pure data parallel, no
collectives.  Compute dtype bf16 (fp32 PSUM accumulation).

Per-core dataflow (all layouts [partition, free]):
  xt       [128, 8kd, 2048]   x^T, bf16 (host pre-transposed)
  per pair: vt [128, 8jt, 512] V^T for 2 slabs = Wv^T @ xt_pair (N=512)
  per slab:
    q_nat/k_nat [128, 2rt, 1024]  rows-on-partitions = xt_slab^T @ W + b
    S psum [64, 64]; softmax on free dim; P^T via 2 PE transposes into a
    [128,64] psum (duplicated on both partition halves), one DVE evac -> wtd
    O^T: per 128-t chunk, TWO quadrant-tiled matmuls read vt directly
      (even j from partitions 0-63 at tile (0,0), odd j from 64-127 at
      (64,64)) -> psum [128, 4t3, 64d] holds an even/odd partition
      permutation of O^T; the permutation is absorbed host-side by
      permuting Wp's rows, so no on-chip interleave copy is needed.
    y [128, 2rt, 1024] = ovt^T @ Wp_perm + bp -> DMA out
Software pipeline (emission order == per-engine issue order): while slab s
runs softmax on DVE/ACT, the PE computes VT of the next pair and Q/K of
slab s+1; the PE transposes for s are emitted mid-way through s+1's
projections.  This keeps TensorE saturated (no >3.4us gaps, HAM stays warm).
"""

import os
import sys

import numpy as np
import ml_dtypes

import concourse.bass as bass
import concourse.mybir as mybir
import concourse.tile as tile
from concourse import bacc
from concourse.bass_utils import run_bass_kernel_spmd


def _install_ntff_hook_shim():
    """concourse's trace path does `from antenv.axon_hooks import
    get_axon_ntff_profile_hook`; this container's antenv lacks that
    module.  Provide it: a ctypes hook on the axon PJRT .so when
    available (mirrors trn_agent_boot), else a None hook (concourse
    then skips tracing gracefully)."""
    try:
        import antenv.axon_hooks  # noqa: F401
        return
    except ImportError:
        pass
    import contextlib
    import ctypes
    import types

    state = {"hook": None}

    def _build_hook():
        so_path = "/opt/axon/libaxon_pjrt.so"
        if not os.path.exists(so_path):
            return None
        lib = ctypes.CDLL(so_path)
        if not hasattr(lib, "axon_start_nrt_profile"):
            return None
        lib.axon_start_nrt_profile.argtypes = [
            ctypes.POINTER(ctypes.c_int64), ctypes.c_size_t]
        lib.axon_start_nrt_profile.restype = ctypes.c_int64
        lib.axon_stop_nrt_profile.argtypes = [ctypes.c_char_p]
        lib.axon_stop_nrt_profile.restype = ctypes.c_int64

        @contextlib.contextmanager
        def _hook(output_dir, device_ids):
            import jax
            jax.devices()
            if device_ids:
                ids = (ctypes.c_int64 * len(device_ids))(*device_ids)
                rc = lib.axon_start_nrt_profile(ids, len(device_ids))
            else:
                rc = lib.axon_start_nrt_profile(None, 0)
            if rc != 0:
                raise RuntimeError(f"axon_start_nrt_profile rc={rc}")
            try:
                yield
            finally:
                n = lib.axon_stop_nrt_profile(str(output_dir).encode())
                if n < 0:
                    raise RuntimeError(f"axon_stop_nrt_profile rc={n}")
                print(f"profile: {n} file(s) written to {output_dir}")

        return _hook

    def get_axon_ntff_profile_hook():
        if state["hook"] is None:
            try:
                state["hook"] = _build_hook()
            except Exception:
                state["hook"] = None
        return state["hook"]

    mod = types.ModuleType("antenv.axon_hooks")
    mod.get_axon_ntff_profile_hook = get_axon_ntff_profile_hook
    mod.set_axon_ntff_profile_hook = lambda h: state.update(hook=h)
    sys.modules["antenv.axon_hooks"] = mod


_install_ntff_hook_shim()

P = 128          # SBUF partitions
DIM = 1024       # model dim
KD = DIM // P    # 8 contraction tiles
ROWS_PER_CORE = 2048
SLABS_PER_CORE = 8
SLAB = 256       # rows per slab
N_CORES = 8
BF16 = mybir.dt.bfloat16
F32 = mybir.dt.float32

_CACHE = {}


def _build_graph():
    nc = bacc.Bacc("TRN2", target_bir_lowering=False, debug=False,
                   num_devices=N_CORES)

    xt_d = nc.dram_tensor("xt", [DIM, ROWS_PER_CORE], BF16, kind="ExternalInput")
    w_d = {
        name: nc.dram_tensor(name, [DIM, DIM], BF16, kind="ExternalInput")
        for name in ("wq", "wk", "wv", "wp")
    }
    bqc_d = nc.dram_tensor("bqc", [P, DIM], BF16, kind="ExternalInput")
    bkc_d = nc.dram_tensor("bkc", [P, DIM], BF16, kind="ExternalInput")
    bpc_d = nc.dram_tensor("bpc", [P, DIM], BF16, kind="ExternalInput")
    bvc_d = nc.dram_tensor("bvc", [P, KD], F32, kind="ExternalInput")
    ident_d = nc.dram_tensor("ident64", [64, 64], BF16, kind="ExternalInput")
    out_d = nc.dram_tensor("out", [ROWS_PER_CORE, DIM], F32, kind="ExternalOutput")

    with tile.TileContext(nc) as tc:
        with (
            tc.tile_pool(name="wpool", bufs=1) as wpool,
            tc.tile_pool(name="xpool", bufs=1) as xpool,
            tc.tile_pool(name="bias", bufs=1) as bias_pool,
            tc.tile_pool(name="qk", bufs=2) as qk_pool,
            tc.tile_pool(name="vt", bufs=2) as vt_pool,
            tc.tile_pool(name="ovt", bufs=2) as ovt_pool,
            tc.tile_pool(name="ysb", bufs=2) as y_pool,
            tc.tile_pool(name="soft", bufs=2) as soft_pool,
            tc.tile_pool(name="ps_proj", bufs=4, space="PSUM") as ps_proj_pool,
            tc.tile_pool(name="ps_s", bufs=1, space="PSUM") as ps_s_pool,
            tc.tile_pool(name="ps_ot", bufs=2, space="PSUM") as ps_ot_pool,
            tc.tile_pool(name="ps_wt", bufs=1, space="PSUM") as ps_wt_pool,
        ):
            # ---- resident tensors / DMA prologue ----------------------------
            # Emission order == per-ring enqueue order: load exactly what the
            # first matmuls need first (wv jt0 pieces + xt cols 0:512), then
            # the rest in order of first use.
            bq_bc = bias_pool.tile([P, DIM], BF16, tag="bqc")
            bk_bc = bias_pool.tile([P, DIM], BF16, tag="bkc")
            bp_bc = bias_pool.tile([P, DIM], BF16, tag="bpc")
            bv_col = bias_pool.tile([P, KD], F32, tag="bvc")
            ident = bias_pool.tile([64, 64], BF16, tag="ident")

            xt_sb = xpool.tile([P, KD, ROWS_PER_CORE], BF16, tag="xt")
            xt_src = xt_d[:].rearrange("(kd p) r -> p kd r", p=P)
            w_sb = {}
            for name in ("wq", "wk", "wv", "wp"):
                w_sb[name] = wpool.tile([P, KD, DIM], BF16, tag=f"w_{name}",
                                        name=f"w_{name}")
            w_srcs = {name: w_d[name][:].rearrange("(kd p) c -> p kd c", p=P)
                      for name in w_sb}

            nc.sync.dma_start(bv_col[:], bvc_d[:])
            nc.sync.dma_start(ident[:], ident_d[:])
            # VT(0) inputs first, interleaved per kd so the first psum group
            # can start as soon as its own pieces land; pieces kept >=1KB
            # per partition row (descriptor-rate, not bandwidth, limits the
            # early load)
            for kd in range(KD):
                nc.sync.dma_start(xt_sb[:, kd, 0:512], xt_src[:, kd, 0:512])
                nc.sync.dma_start(w_sb["wv"][:, kd, :], w_srcs["wv"][:, kd, :])
            # then in order of first use
            for kd in range(KD):
                nc.sync.dma_start(w_sb["wq"][:, kd, 0:512],
                                  w_srcs["wq"][:, kd, 0:512])
            nc.sync.dma_start(bq_bc[:], bqc_d[:])
            for kd in range(KD):
                nc.sync.dma_start(w_sb["wk"][:, kd, 0:512],
                                  w_srcs["wk"][:, kd, 0:512])
            nc.sync.dma_start(bk_bc[:], bkc_d[:])
            for name in ("wq", "wk"):
                for kd in range(KD):
                    nc.sync.dma_start(w_sb[name][:, kd, 512:DIM],
                                      w_srcs[name][:, kd, 512:DIM])
            nc.sync.dma_start(bp_bc[:], bpc_d[:])
            for jc in range(2):
                for kd in range(KD):
                    nc.sync.dma_start(
                        w_sb["wp"][:, kd, jc * 512:(jc + 1) * 512],
                        w_srcs["wp"][:, kd, jc * 512:(jc + 1) * 512])
            # remaining xt (pairs 1-3)
            for half in range(1, 4):
                for kd in range(KD):
                    nc.sync.dma_start(
                        xt_sb[:, kd, half * 512:(half + 1) * 512],
                        xt_src[:, kd, half * 512:(half + 1) * 512])

            # ---- stage emitters ---------------------------------------------
            def emit_vt(pair):
                """V^T for both slabs of the pair (N=512 streams)."""
                vtt = vt_pool.tile([P, KD, 2 * SLAB], BF16, tag="vt")
                p0 = pair * 2 * SLAB
                for jt in range(KD):
                    ps = ps_proj_pool.tile([P, 512], F32, tag="ps_proj")
                    for kd in range(KD):
                        nc.tensor.matmul(
                            ps[:],
                            w_sb["wv"][:, kd, jt * P:(jt + 1) * P],
                            xt_sb[:, kd, p0: p0 + 512],
                            start=(kd == 0),
                            stop=(kd == KD - 1),
                        )
                    nc.scalar.activation(
                        vtt[:, jt, :], ps[:],
                        mybir.ActivationFunctionType.Identity,
                        bias=bv_col[:, jt: jt + 1])
                return vtt

            def emit_proj(s, dst_t, wname, bias_bc):
                """One of Q/K natural-layout projections for slab s."""
                c0 = s * SLAB
                for rt in range(2):
                    for jc in range(2):
                        ps = ps_proj_pool.tile([P, 512], F32, tag="ps_proj")
                        for kd in range(KD):
                            nc.tensor.matmul(
                                ps[:],
                                xt_sb[:, kd,
                                      c0 + rt * P: c0 + (rt + 1) * P],
                                w_sb[wname][:, kd,
                                            jc * 512:(jc + 1) * 512],
                                start=(kd == 0),
                                stop=(kd == KD - 1),
                            )
                        nc.vector.tensor_add(
                            dst_t[:, rt, jc * 512:(jc + 1) * 512],
                            ps[:],
                            bias_bc[:, jc * 512:(jc + 1) * 512],
                        )

            def emit_s_mms(q_nat, k_nat):
                """S = sum over (rt, t2) of Q_blk^T @ K_blk -> PSUM [64,64]."""
                ps_s = ps_s_pool.tile([64, 64], F32, tag="ps_s")
                n_acc = 0
                for rt in range(2):
                    for t2 in range(16):
                        nc.tensor.matmul(
                            ps_s[:],
                            q_nat[:, rt, t2 * 64:(t2 + 1) * 64],
                            k_nat[:, rt, t2 * 64:(t2 + 1) * 64],
                            start=(n_acc == 0),
                            stop=(n_acc == 31),
                        )
                        n_acc += 1
                return ps_s

            def emit_softmax(ps_s):
                """softmax over the free dim (DVE/ACT only)."""
                negmax = soft_pool.tile([64, 1], F32, tag="negmax")
                nc.vector.reduce_max(negmax[:], ps_s[:],
                                     axis=mybir.AxisListType.X, negate=True)
                p_sb = soft_pool.tile([64, 64], F32, tag="p_sb")
                rsum = soft_pool.tile([64, 1], F32, tag="rsum")
                nc.scalar.activation(p_sb[:], ps_s[:],
                                     mybir.ActivationFunctionType.Exp,
                                     bias=negmax[:], accum_out=rsum[:])
                rinv = soft_pool.tile([64, 1], F32, tag="rinv")
                nc.vector.reciprocal(rinv[:], rsum[:])
                w_soft = soft_pool.tile([64, 64], BF16, tag="w_soft")
                nc.vector.tensor_scalar_mul(w_soft[:], p_sb[:], rinv[:])
                return w_soft

            def emit_wt(w_soft):
                """P^T duplicated on both partition halves: two quadrant
                PE transposes into one [128,64] psum, one DVE evac."""
                ps_wt = ps_wt_pool.tile([P, 64], BF16, tag="ps_wt")
                nc.tensor.transpose(ps_wt[0:64, :], w_soft[:], ident[:])
                nc.tensor.transpose(ps_wt[64:128, :], w_soft[:], ident[:])
                wtd = soft_pool.tile([P, 64], BF16, tag="wtd")
                nc.vector.tensor_copy(wtd[:], ps_wt[:])
                return wtd

            def emit_ot(s, vtt, wtd):
                """O^T straight from vt, chunked by (g=jt, t3, parity): each
                stationary is a contiguous 64-col slice of vt; per (g,t3) the
                two parities run on disjoint array quadrants concurrently.
                The resulting psum partition scatter (c_dim = 16*(q%64) + 2g
                + (q>=64)) is absorbed into Wp's host-side row order."""
                half = s % 2
                base = half * SLAB
                ovt = ovt_pool.tile([P, KD, SLAB], BF16, tag="ovt")
                for g in range(KD):
                    pso = ps_ot_pool.tile([P, 4, 64], F32, tag="ps_ot")
                    for t3 in range(4):
                        col = base + 64 * t3
                        nc.tensor.matmul(pso[0:64, t3, :],
                                         vtt[0:64, g, col:col + 64],
                                         wtd[0:64, :], start=True, stop=True)
                        nc.tensor.matmul(pso[64:128, t3, :],
                                         vtt[64:128, g, col:col + 64],
                                         wtd[64:128, :], start=True, stop=True)
                    nc.vector.tensor_copy(
                        ovt[:, g, :].rearrange("p (d four) -> p d four",
                                               four=4),
                        pso[:].rearrange("p t3 d -> p d t3"),
                    )
                return ovt

            def emit_y(s, ovt):
                """Y = OvT^T @ Wp_perm + bp (natural rows) -> DMA out."""
                y_sb = y_pool.tile([P, 2, DIM], F32, tag="y_sb")
                for rt in range(2):
                    for jc in range(2):
                        ps = ps_proj_pool.tile([P, 512], F32, tag="ps_proj")
                        for ct in range(KD):
                            nc.tensor.matmul(
                                ps[:],
                                ovt[:, ct, rt * P:(rt + 1) * P],
                                w_sb["wp"][:, ct, jc * 512:(jc + 1) * 512],
                                start=(ct == 0),
                                stop=(ct == KD - 1),
                            )
                        nc.vector.tensor_add(
                            y_sb[:, rt, jc * 512:(jc + 1) * 512],
                            ps[:],
                            bp_bc[:, jc * 512:(jc + 1) * 512],
                        )
                out_dst = out_d[s * SLAB:(s + 1) * SLAB, :] \
                    .rearrange("(rt p) c -> p rt c", p=P)
                for rt in range(2):
                    for jc in range(2):
                        nc.sync.dma_start(
                            out_dst[:, rt, jc * 512:(jc + 1) * 512],
                            y_sb[:, rt, jc * 512:(jc + 1) * 512])

            # ---- software pipeline ------------------------------------------
            vt_tiles = {0: emit_vt(0)}
            q_nat = qk_pool.tile([P, 2, DIM], BF16, tag="q_nat")
            k_nat = qk_pool.tile([P, 2, DIM], BF16, tag="k_nat")
            emit_proj(0, q_nat, "wq", bq_bc)
            emit_proj(0, k_nat, "wk", bk_bc)

            for s in range(SLABS_PER_CORE):
                pair = s // 2
                ps_s = emit_s_mms(q_nat, k_nat)
                w_soft = emit_softmax(ps_s)
                if s % 2 == 1 and pair + 1 < SLABS_PER_CORE // 2:
                    vt_tiles[pair + 1] = emit_vt(pair + 1)
                if s + 1 < SLABS_PER_CORE:
                    q_next = qk_pool.tile([P, 2, DIM], BF16, tag="q_nat")
                    k_next = qk_pool.tile([P, 2, DIM], BF16, tag="k_nat")
                    emit_proj(s + 1, q_next, "wq", bq_bc)
                else:
                    q_next = k_next = None
                wtd = emit_wt(w_soft)
                if k_next is not None:
                    emit_proj(s + 1, k_next, "wk", bk_bc)
                ovt = emit_ot(s, vt_tiles[pair], wtd)
                emit_y(s, ovt)
                q_nat, k_nat = q_next, k_next

    nc.compile()
    return nc


def _prep_inputs(x, Wq, bq, Wk, bk, Wv, bv, Wp, bp):
    """Host-side shard prep. Returns in_maps list for 8 cores."""
    bf16 = ml_dtypes.bfloat16
    xf = np.ascontiguousarray(np.asarray(x, dtype=np.float32).reshape(-1, DIM))
    scale = np.float32(1.0 / np.sqrt(64.0))

    wq_b = np.ascontiguousarray((np.asarray(Wq) * scale).astype(bf16))
    wk_b = np.ascontiguousarray(np.asarray(Wk).astype(bf16))
    wv_b = np.ascontiguousarray(np.asarray(Wv).astype(bf16))
    # Absorb the O^T psum partition scatter into Wp's rows: contraction
    # chunk g, psum partition q holds O_view^T row c = 16*(q%64)+2g+(q>=64).
    q_idx = np.arange(P)
    src_rows = (16 * (q_idx % 64)[None, :] + 2 * np.arange(KD)[:, None]
                + (q_idx >= 64)[None, :])                 # [KD, P]
    wp_b = np.asarray(Wp).astype(bf16)[src_rows.reshape(-1), :]
    wp_b = np.ascontiguousarray(wp_b)

    bqc = np.ascontiguousarray(np.broadcast_to(
        (np.asarray(bq) * scale).astype(bf16), (P, DIM)))
    bkc = np.ascontiguousarray(np.broadcast_to(
        np.asarray(bk).astype(bf16), (P, DIM)))
    bpc = np.ascontiguousarray(np.broadcast_to(
        np.asarray(bp).astype(bf16), (P, DIM)))
    bvc = np.ascontiguousarray(
        np.asarray(bv, dtype=np.float32).reshape(KD, P).T)
    ident = np.eye(64, dtype=bf16)

    shared = {
        "wq": wq_b, "wk": wk_b, "wv": wv_b, "wp": wp_b,
        "bqc": bqc, "bkc": bkc, "bpc": bpc, "bvc": bvc,
        "ident64": ident,
    }
    in_maps = []
    for c in range(N_CORES):
        xs = xf[c * ROWS_PER_CORE:(c + 1) * ROWS_PER_CORE]  # [2048, 1024]
        xt = np.ascontiguousarray(xs.T.astype(bf16))        # [1024, 2048]
        in_maps.append({"xt": xt, **shared})
    return in_maps


def kernel(x, Wq, bq, Wk, bk, Wv, bv, Wp, bp):
    if "nc" not in _CACHE:
        _CACHE["nc"] = _build_graph()
    nc = _CACHE["nc"]

    in_maps = _prep_inputs(x, Wq, bq, Wk, bk, Wv, bv, Wp, bp)
    trace = bool(int(os.environ.get("ATHENA_TRACE", "0")))
    res = run_bass_kernel_spmd(nc, in_maps, core_ids=list(range(N_CORES)),
                               trace=trace)
    _CACHE["last_exec_time_ns"] = res.exec_time_ns

    out = np.concatenate([res.results[c]["out"] for c in range(N_CORES)], axis=0)
    return np.ascontiguousarray(out.reshape(np.asarray(x).shape)
                                .astype(np.float32))


# revision 15
# speedup vs baseline: 1.2035x; 1.2035x over previous
"""Trainium2 Bass kernel for nn_Attention_1580547974448.

Math insight: the reference uses raw .reshape (not a head-split transpose) on
[B,T,H*HD] -> [B,H,T,HD].  With B=4, T=4096, DIM=1024, H=16, HD=64 this makes
each "head" a contiguous 256-row slab of the flattened [B*T, DIM] = [16384,1024]
input: for slab s (rows 256s..256s+255),
    Q = (x_s @ Wq + bq)            viewed row-major as [4096, 64]
    S = Q^T K / sqrt(64)           [64, 64]
    P = softmax(S, axis=-1)
    O = P @ V^T                    [64, 4096], row-major == [256, 1024]
    y_s = O_v @ Wp + bp
i.e. the whole computation is block-diagonal over 64 independent slabs.
We shard 8 slabs (2048 rows) per NeuronCore -> pure data parallel, no
collectives.  Compute dtype bf16 (fp32 PSUM accumulation).

Per-core dataflow (all layouts [partition, free]):
  xt       [128, 8kd, 2048]   x^T, bf16 (host pre-transposed)
  per pair: vt [128, 8jt, 512] V^T for 2 slabs = Wv^T @ xt_pair (N=512)
  per slab:
    q_nat/k_nat [128, 2rt, 1024]  rows-on-partitions = xt_slab^T @ W + b
    S psum [64, 64]; softmax on free dim; P^T via 2 PE transposes into a
    [128,64] psum (duplicated on both partition halves), one DVE evac -> wtd
    O^T: per 128-t chunk, TWO quadrant-tiled matmuls read vt directly
      (even j from partitions 0-63 at tile (0,0), odd j from 64-127 at
      (64,64)) -> psum [128, 4t3, 64d] holds an even/odd partition
      permutation of O^T; the permutation is absorbed host-side by
      permuting Wp's rows, so no on-chip interleave copy is needed.
    y [128, 2rt, 1024] = ovt^T @ Wp_perm + bp -> DMA out
Software pipeline (emission order == per-engine issue order): while slab s
runs softmax on DVE/ACT, the PE computes VT of the next pair and Q/K of
slab s+1; the PE transposes for s are emitted mid-way through s+1's
projections.  This keeps TensorE saturated (no >3.4us gaps, HAM stays warm).
"""

import os
import sys

import numpy as np
import ml_dtypes

import concourse.bass as bass
import concourse.mybir as mybir
import concourse.tile as tile
from concourse import bacc
from concourse.bass_utils import run_bass_kernel_spmd


def _install_ntff_hook_shim():
    """concourse's trace path does `from antenv.axon_hooks import
    get_axon_ntff_profile_hook`; this container's antenv lacks that
    module.  Provide it: a ctypes hook on the axon PJRT .so when
    available (mirrors trn_agent_boot), else a None hook (concourse
    then skips tracing gracefully)."""
    try:
        import antenv.axon_hooks  # noqa: F401
        return
    except ImportError:
        pass
    import contextlib
    import ctypes
    import types

    state = {"hook": None}

    def _build_hook():
        so_path = "/opt/axon/libaxon_pjrt.so"
        if not os.path.exists(so_path):
            return None
        lib = ctypes.CDLL(so_path)
        if not hasattr(lib, "axon_start_nrt_profile"):
            return None
        lib.axon_start_nrt_profile.argtypes = [
            ctypes.POINTER(ctypes.c_int64), ctypes.c_size_t]
        lib.axon_start_nrt_profile.restype = ctypes.c_int64
        lib.axon_stop_nrt_profile.argtypes = [ctypes.c_char_p]
        lib.axon_stop_nrt_profile.restype = ctypes.c_int64

        @contextlib.contextmanager
        def _hook(output_dir, device_ids):
            import jax
            jax.devices()
            if device_ids:
                ids = (ctypes.c_int64 * len(device_ids))(*device_ids)
                rc = lib.axon_start_nrt_profile(ids, len(device_ids))
            else:
                rc = lib.axon_start_nrt_profile(None, 0)
            if rc != 0:
                raise RuntimeError(f"axon_start_nrt_profile rc={rc}")
            try:
                yield
            finally:
                n = lib.axon_stop_nrt_profile(str(output_dir).encode())
                if n < 0:
                    raise RuntimeError(f"axon_stop_nrt_profile rc={n}")
                print(f"profile: {n} file(s) written to {output_dir}")

        return _hook

    def get_axon_ntff_profile_hook():
        if state["hook"] is None:
            try:
                state["hook"] = _build_hook()
            except Exception:
                state["hook"] = None
        return state["hook"]

    mod = types.ModuleType("antenv.axon_hooks")
    mod.get_axon_ntff_profile_hook = get_axon_ntff_profile_hook
    mod.set_axon_ntff_profile_hook = lambda h: state.update(hook=h)
    sys.modules["antenv.axon_hooks"] = mod


_install_ntff_hook_shim()

P = 128          # SBUF partitions
DIM = 1024       # model dim
KD = DIM // P    # 8 contraction tiles
ROWS_PER_CORE = 2048
SLABS_PER_CORE = 8
SLAB = 256       # rows per slab
N_CORES = 8
BF16 = mybir.dt.bfloat16
F32 = mybir.dt.float32

_CACHE = {}


def _build_graph():
    nc = bacc.Bacc("TRN2", target_bir_lowering=False, debug=False,
                   num_devices=N_CORES)

    xt_d = nc.dram_tensor("xt", [DIM, ROWS_PER_CORE], BF16, kind="ExternalInput")
    w_d = {
        name: nc.dram_tensor(name, [DIM, DIM], BF16, kind="ExternalInput")
        for name in ("wq", "wk", "wv", "wp")
    }
    bqc_d = nc.dram_tensor("bqc", [P, DIM], BF16, kind="ExternalInput")
    bkc_d = nc.dram_tensor("bkc", [P, DIM], BF16, kind="ExternalInput")
    bpc_d = nc.dram_tensor("bpc", [P, DIM], BF16, kind="ExternalInput")
    bvc_d = nc.dram_tensor("bvc", [P, KD], F32, kind="ExternalInput")
    ident_d = nc.dram_tensor("ident64", [64, 64], BF16, kind="ExternalInput")
    out_d = nc.dram_tensor("out", [ROWS_PER_CORE, DIM], F32, kind="ExternalOutput")

    with tile.TileContext(nc) as tc:
        with (
            tc.tile_pool(name="wpool", bufs=1) as wpool,
            tc.tile_pool(name="xpool", bufs=1) as xpool,
            tc.tile_pool(name="bias", bufs=1) as bias_pool,
            tc.tile_pool(name="qk", bufs=2) as qk_pool,
            tc.tile_pool(name="vt", bufs=2) as vt_pool,
            tc.tile_pool(name="ovt", bufs=2) as ovt_pool,
            tc.tile_pool(name="ysb", bufs=2) as y_pool,
            tc.tile_pool(name="soft", bufs=2) as soft_pool,
            tc.tile_pool(name="ps_proj", bufs=4, space="PSUM") as ps_proj_pool,
            tc.tile_pool(name="ps_s", bufs=1, space="PSUM") as ps_s_pool,
            tc.tile_pool(name="ps_ot", bufs=2, space="PSUM") as ps_ot_pool,
            tc.tile_pool(name="ps_wt", bufs=1, space="PSUM") as ps_wt_pool,
        ):
            # ---- resident tensors / DMA prologue ----------------------------
            # Emission order == per-ring enqueue order: load exactly what the
            # first matmuls need first (wv jt0 pieces + xt cols 0:512), then
            # the rest in order of first use.
            bq_bc = bias_pool.tile([P, DIM], BF16, tag="bqc")
            bk_bc = bias_pool.tile([P, DIM], BF16, tag="bkc")
            bp_bc = bias_pool.tile([P, DIM], BF16, tag="bpc")
            bv_col = bias_pool.tile([P, KD], F32, tag="bvc")
            ident = bias_pool.tile([64, 64], BF16, tag="ident")

            xt_sb = xpool.tile([P, KD, ROWS_PER_CORE], BF16, tag="xt")
            xt_src = xt_d[:].rearrange("(kd p) r -> p kd r", p=P)
            w_sb = {}
            for name in ("wq", "wk", "wv", "wp"):
                w_sb[name] = wpool.tile([P, KD, DIM], BF16, tag=f"w_{name}",
                                        name=f"w_{name}")
            w_srcs = {name: w_d[name][:].rearrange("(kd p) c -> p kd c", p=P)
                      for name in w_sb}

            # HAM pre-warm: ~12 junk matmuls on a memset tile keep the PE
            # busy from ~6us so the clock gate opens before real data lands
            warm = bias_pool.tile([P, 512], BF16, tag="warm")
            nc.vector.memset(warm[:], 0.0)
            ps_warm = ps_proj_pool.tile([P, 512], F32, tag="ps_proj")
            for i in range(12):
                nc.tensor.matmul(ps_warm[:], warm[:, 0:P], warm[:],
                                 start=(i == 0), stop=(i == 11))

            nc.sync.dma_start(bv_col[:], bvc_d[:])
            nc.sync.dma_start(ident[:], ident_d[:])
            # VT(0) inputs first, interleaved per kd so the first psum group
            # can start as soon as its own pieces land; pieces kept >=1KB
            # per partition row (descriptor-rate, not bandwidth, limits the
            # early load)
            for kd in range(KD):
                nc.sync.dma_start(xt_sb[:, kd, 0:512], xt_src[:, kd, 0:512])
                nc.sync.dma_start(w_sb["wv"][:, kd, :], w_srcs["wv"][:, kd, :])
            # then in order of first use
            for kd in range(KD):
                nc.sync.dma_start(w_sb["wq"][:, kd, 0:512],
                                  w_srcs["wq"][:, kd, 0:512])
            nc.sync.dma_start(bq_bc[:], bqc_d[:])
            for kd in range(KD):
                nc.sync.dma_start(w_sb["wk"][:, kd, 0:512],
                                  w_srcs["wk"][:, kd, 0:512])
            nc.sync.dma_start(bk_bc[:], bkc_d[:])
            for name in ("wq", "wk"):
                for kd in range(KD):
                    nc.sync.dma_start(w_sb[name][:, kd, 512:DIM],
                                      w_srcs[name][:, kd, 512:DIM])
            nc.sync.dma_start(bp_bc[:], bpc_d[:])
            for jc in range(2):
                for kd in range(KD):
                    nc.sync.dma_start(
                        w_sb["wp"][:, kd, jc * 512:(jc + 1) * 512],
                        w_srcs["wp"][:, kd, jc * 512:(jc + 1) * 512])
            # remaining xt (pairs 1-3)
            for half in range(1, 4):
                for kd in range(KD):
                    nc.sync.dma_start(
                        xt_sb[:, kd, half * 512:(half + 1) * 512],
                        xt_src[:, kd, half * 512:(half + 1) * 512])

            # ---- stage emitters ---------------------------------------------
            def emit_vt(pair):
                """V^T for both slabs of the pair (N=512 streams)."""
                vtt = vt_pool.tile([P, KD, 2 * SLAB], BF16, tag="vt")
                p0 = pair * 2 * SLAB
                for jt in range(KD):
                    ps = ps_proj_pool.tile([P, 512], F32, tag="ps_proj")
                    for kd in range(KD):
                        nc.tensor.matmul(
                            ps[:],
                            w_sb["wv"][:, kd, jt * P:(jt + 1) * P],
                            xt_sb[:, kd, p0: p0 + 512],
                            start=(kd == 0),
                            stop=(kd == KD - 1),
                        )
                    nc.scalar.activation(
                        vtt[:, jt, :], ps[:],
                        mybir.ActivationFunctionType.Identity,
                        bias=bv_col[:, jt: jt + 1])
                return vtt

            def emit_proj(s, dst_t, wname, bias_bc):
                """One of Q/K natural-layout projections for slab s."""
                c0 = s * SLAB
                for rt in range(2):
                    for jc in range(2):
                        ps = ps_proj_pool.tile([P, 512], F32, tag="ps_proj")
                        for kd in range(KD):
                            nc.tensor.matmul(
                                ps[:],
                                xt_sb[:, kd,
                                      c0 + rt * P: c0 + (rt + 1) * P],
                                w_sb[wname][:, kd,
                                            jc * 512:(jc + 1) * 512],
                                start=(kd == 0),
                                stop=(kd == KD - 1),
                            )
                        nc.vector.tensor_add(
                            dst_t[:, rt, jc * 512:(jc + 1) * 512],
                            ps[:],
                            bias_bc[:, jc * 512:(jc + 1) * 512],
                        )

            def emit_s_mms(q_nat, k_nat):
                """S = sum over (rt, t2) of Q_blk^T @ K_blk -> PSUM [64,64]."""
                ps_s = ps_s_pool.tile([64, 64], F32, tag="ps_s")
                n_acc = 0
                for rt in range(2):
                    for t2 in range(16):
                        nc.tensor.matmul(
                            ps_s[:],
                            q_nat[:, rt, t2 * 64:(t2 + 1) * 64],
                            k_nat[:, rt, t2 * 64:(t2 + 1) * 64],
                            start=(n_acc == 0),
                            stop=(n_acc == 31),
                        )
                        n_acc += 1
                return ps_s

            def emit_softmax(ps_s):
                """softmax over the free dim (DVE/ACT only)."""
                negmax = soft_pool.tile([64, 1], F32, tag="negmax")
                nc.vector.reduce_max(negmax[:], ps_s[:],
                                     axis=mybir.AxisListType.X, negate=True)
                p_sb = soft_pool.tile([64, 64], F32, tag="p_sb")
                rsum = soft_pool.tile([64, 1], F32, tag="rsum")
                nc.scalar.activation(p_sb[:], ps_s[:],
                                     mybir.ActivationFunctionType.Exp,
                                     bias=negmax[:], accum_out=rsum[:])
                rinv = soft_pool.tile([64, 1], F32, tag="rinv")
                nc.vector.reciprocal(rinv[:], rsum[:])
                w_soft = soft_pool.tile([64, 64], BF16, tag="w_soft")
                nc.vector.tensor_scalar_mul(w_soft[:], p_sb[:], rinv[:])
                return w_soft

            def emit_wt(w_soft):
                """P^T duplicated on both partition halves: two quadrant
                PE transposes into one [128,64] psum, one DVE evac."""
                ps_wt = ps_wt_pool.tile([P, 64], BF16, tag="ps_wt")
                nc.tensor.transpose(ps_wt[0:64, :], w_soft[:], ident[:])
                nc.tensor.transpose(ps_wt[64:128, :], w_soft[:], ident[:])
                wtd = soft_pool.tile([P, 64], BF16, tag="wtd")
                nc.vector.tensor_copy(wtd[:], ps_wt[:])
                return wtd

            def emit_ot(s, vtt, wtd):
                """O^T straight from vt, chunked by (g=jt, t3, parity): each
                stationary is a contiguous 64-col slice of vt; per (g,t3) the
                two parities run on disjoint array quadrants concurrently.
                The resulting psum partition scatter (c_dim = 16*(q%64) + 2g
                + (q>=64)) is absorbed into Wp's host-side row order."""
                half = s % 2
                base = half * SLAB
                ovt = ovt_pool.tile([P, KD, SLAB], BF16, tag="ovt")
                for g in range(KD):
                    pso = ps_ot_pool.tile([P, 4, 64], F32, tag="ps_ot")
                    for t3 in range(4):
                        col = base + 64 * t3
                        nc.tensor.matmul(pso[0:64, t3, :],
                                         vtt[0:64, g, col:col + 64],
                                         wtd[0:64, :], start=True, stop=True)
                        nc.tensor.matmul(pso[64:128, t3, :],
                                         vtt[64:128, g, col:col + 64],
                                         wtd[64:128, :], start=True, stop=True)
                    nc.vector.tensor_copy(
                        ovt[:, g, :].rearrange("p (d four) -> p d four",
                                               four=4),
                        pso[:].rearrange("p t3 d -> p d t3"),
                    )
                return ovt

            def emit_y(s, ovt):
                """Y = OvT^T @ Wp_perm + bp (natural rows) -> DMA out."""
                y_sb = y_pool.tile([P, 2, DIM], F32, tag="y_sb")
                for rt in range(2):
                    for jc in range(2):
                        ps = ps_proj_pool.tile([P, 512], F32, tag="ps_proj")
                        for ct in range(KD):
                            nc.tensor.matmul(
                                ps[:],
                                ovt[:, ct, rt * P:(rt + 1) * P],
                                w_sb["wp"][:, ct, jc * 512:(jc + 1) * 512],
                                start=(ct == 0),
                                stop=(ct == KD - 1),
                            )
                        nc.vector.tensor_add(
                            y_sb[:, rt, jc * 512:(jc + 1) * 512],
                            ps[:],
                            bp_bc[:, jc * 512:(jc + 1) * 512],
                        )
                out_dst = out_d[s * SLAB:(s + 1) * SLAB, :] \
                    .rearrange("(rt p) c -> p rt c", p=P)
                for rt in range(2):
                    for jc in range(2):
                        nc.sync.dma_start(
                            out_dst[:, rt, jc * 512:(jc + 1) * 512],
                            y_sb[:, rt, jc * 512:(jc + 1) * 512])

            # ---- software pipeline ------------------------------------------
            vt_tiles = {0: emit_vt(0)}
            q_nat = qk_pool.tile([P, 2, DIM], BF16, tag="q_nat")
            k_nat = qk_pool.tile([P, 2, DIM], BF16, tag="k_nat")
            emit_proj(0, q_nat, "wq", bq_bc)
            emit_proj(0, k_nat, "wk", bk_bc)

            last = SLABS_PER_CORE - 1
            w_soft_next = None
            for s in range(SLABS_PER_CORE):
                pair = s // 2
                if w_soft_next is None:
                    ps_s = emit_s_mms(q_nat, k_nat)
                    w_soft = emit_softmax(ps_s)
                else:
                    w_soft = w_soft_next
                if s % 2 == 1 and pair + 1 < SLABS_PER_CORE // 2:
                    vt_tiles[pair + 1] = emit_vt(pair + 1)
                if s + 1 < SLABS_PER_CORE:
                    q_next = qk_pool.tile([P, 2, DIM], BF16, tag="q_nat")
                    k_next = qk_pool.tile([P, 2, DIM], BF16, tag="k_nat")
                    emit_proj(s + 1, q_next, "wq", bq_bc)
                else:
                    q_next = k_next = None
                wtd = emit_wt(w_soft)
                if k_next is not None:
                    emit_proj(s + 1, k_next, "wk", bk_bc)
                if s + 1 == last:
                    # hoist the last slab's S + softmax here so its serial
                    # DVE/ACT chain overlaps E(s)/F(s) instead of stalling
                    # the PE at the very end
                    ps_s7 = emit_s_mms(q_next, k_next)
                    w_soft_next = emit_softmax(ps_s7)
                ovt = emit_ot(s, vt_tiles[pair], wtd)
                emit_y(s, ovt)
                q_nat, k_nat = q_next, k_next

    nc.compile()
    return nc


def _prep_inputs(x, Wq, bq, Wk, bk, Wv, bv, Wp, bp):
    """Host-side shard prep. Returns in_maps list for 8 cores."""
    bf16 = ml_dtypes.bfloat16
    xf = np.ascontiguousarray(np.asarray(x, dtype=np.float32).reshape(-1, DIM))
    scale = np.float32(1.0 / np.sqrt(64.0))

    wq_b = np.ascontiguousarray((np.asarray(Wq) * scale).astype(bf16))
    wk_b = np.ascontiguousarray(np.asarray(Wk).astype(bf16))
    wv_b = np.ascontiguousarray(np.asarray(Wv).astype(bf16))
    # Absorb the O^T psum partition scatter into Wp's rows: contraction
    # chunk g, psum partition q holds O_view^T row c = 16*(q%64)+2g+(q>=64).
    q_idx = np.arange(P)
    src_rows = (16 * (q_idx % 64)[None, :] + 2 * np.arange(KD)[:, None]
                + (q_idx >= 64)[None, :])                 # [KD, P]
    wp_b = np.asarray(Wp).astype(bf16)[src_rows.reshape(-1), :]
    wp_b = np.ascontiguousarray(wp_b)

    bqc = np.ascontiguousarray(np.broadcast_to(
        (np.asarray(bq) * scale).astype(bf16), (P, DIM)))
    bkc = np.ascontiguousarray(np.broadcast_to(
        np.asarray(bk).astype(bf16), (P, DIM)))
    bpc = np.ascontiguousarray(np.broadcast_to(
        np.asarray(bp).astype(bf16), (P, DIM)))
    bvc = np.ascontiguousarray(
        np.asarray(bv, dtype=np.float32).reshape(KD, P).T)
    ident = np.eye(64, dtype=bf16)

    shared = {
        "wq": wq_b, "wk": wk_b, "wv": wv_b, "wp": wp_b,
        "bqc": bqc, "bkc": bkc, "bpc": bpc, "bvc": bvc,
        "ident64": ident,
    }
    in_maps = []
    for c in range(N_CORES):
        xs = xf[c * ROWS_PER_CORE:(c + 1) * ROWS_PER_CORE]  # [2048, 1024]
        xt = np.ascontiguousarray(xs.T.astype(bf16))        # [1024, 2048]
        in_maps.append({"xt": xt, **shared})
    return in_maps


def kernel(x, Wq, bq, Wk, bk, Wv, bv, Wp, bp):
    if "nc" not in _CACHE:
        _CACHE["nc"] = _build_graph()
    nc = _CACHE["nc"]

    in_maps = _prep_inputs(x, Wq, bq, Wk, bk, Wv, bv, Wp, bp)
    trace = bool(int(os.environ.get("ATHENA_TRACE", "0")))
    res = run_bass_kernel_spmd(nc, in_maps, core_ids=list(range(N_CORES)),
                               trace=trace)
    _CACHE["last_exec_time_ns"] = res.exec_time_ns

    out = np.concatenate([res.results[c]["out"] for c in range(N_CORES)], axis=0)
    return np.ascontiguousarray(out.reshape(np.asarray(x).shape)
                                .astype(np.float32))


# revision 18
# speedup vs baseline: 1.2280x; 1.0203x over previous
"""Trainium2 Bass kernel for nn_Attention_1580547974448.

Math insight: the reference uses raw .reshape (not a head-split transpose) on
[B,T,H*HD] -> [B,H,T,HD].  With B=4, T=4096, DIM=1024, H=16, HD=64 this makes
each "head" a contiguous 256-row slab of the flattened [B*T, DIM] = [16384,1024]
input: for slab s (rows 256s..256s+255),
    Q = (x_s @ Wq + bq)            viewed row-major as [4096, 64]
    S = Q^T K / sqrt(64)           [64, 64]
    P = softmax(S, axis=-1)
    O = P @ V^T                    [64, 4096], row-major == [256, 1024]
    y_s = O_v @ Wp + bp
i.e. the whole computation is block-diagonal over 64 independent slabs.
We shard 8 slabs (2048 rows) per NeuronCore -> pure data parallel, no
collectives.  Compute dtype bf16 (fp32 PSUM accumulation).

Per-core dataflow (all layouts [partition, free]):
  xt       [128, 8kd, 2048]   x^T, bf16 (host pre-transposed)
  per pair: vt [128, 8jt, 512] V^T for 2 slabs = Wv^T @ xt_pair (N=512)
  per slab:
    q_nat/k_nat [128, 2rt, 1024]  rows-on-partitions = xt_slab^T @ W + b
    S psum [64, 64]; softmax on free dim; P^T via 2 PE transposes into a
    [128,64] psum (duplicated on both partition halves), one DVE evac -> wtd
    O^T: per 128-t chunk, TWO quadrant-tiled matmuls read vt directly
      (even j from partitions 0-63 at tile (0,0), odd j from 64-127 at
      (64,64)) -> psum [128, 4t3, 64d] holds an even/odd partition
      permutation of O^T; the permutation is absorbed host-side by
      permuting Wp's rows, so no on-chip interleave copy is needed.
    y [128, 2rt, 1024] = ovt^T @ Wp_perm + bp -> DMA out
Software pipeline (emission order == per-engine issue order): while slab s
runs softmax on DVE/ACT, the PE computes VT of the next pair and Q/K of
slab s+1; the PE transposes for s are emitted mid-way through s+1's
projections.  This keeps TensorE saturated (no >3.4us gaps, HAM stays warm).
"""

import os
import sys

import numpy as np
import ml_dtypes

import concourse.bass as bass
import concourse.mybir as mybir
import concourse.tile as tile
from concourse import bacc
from concourse.bass_utils import run_bass_kernel_spmd


def _install_ntff_hook_shim():
    """concourse's trace path does `from antenv.axon_hooks import
    get_axon_ntff_profile_hook`; this container's antenv lacks that
    module.  Provide it: a ctypes hook on the axon PJRT .so when
    available (mirrors trn_agent_boot), else a None hook (concourse
    then skips tracing gracefully)."""
    try:
        import antenv.axon_hooks  # noqa: F401
        return
    except ImportError:
        pass
    import contextlib
    import ctypes
    import types

    state = {"hook": None}

    def _build_hook():
        so_path = "/opt/axon/libaxon_pjrt.so"
        if not os.path.exists(so_path):
            return None
        lib = ctypes.CDLL(so_path)
        if not hasattr(lib, "axon_start_nrt_profile"):
            return None
        lib.axon_start_nrt_profile.argtypes = [
            ctypes.POINTER(ctypes.c_int64), ctypes.c_size_t]
        lib.axon_start_nrt_profile.restype = ctypes.c_int64
        lib.axon_stop_nrt_profile.argtypes = [ctypes.c_char_p]
        lib.axon_stop_nrt_profile.restype = ctypes.c_int64

        @contextlib.contextmanager
        def _hook(output_dir, device_ids):
            import jax
            jax.devices()
            if device_ids:
                ids = (ctypes.c_int64 * len(device_ids))(*device_ids)
                rc = lib.axon_start_nrt_profile(ids, len(device_ids))
            else:
                rc = lib.axon_start_nrt_profile(None, 0)
            if rc != 0:
                raise RuntimeError(f"axon_start_nrt_profile rc={rc}")
            try:
                yield
            finally:
                n = lib.axon_stop_nrt_profile(str(output_dir).encode())
                if n < 0:
                    raise RuntimeError(f"axon_stop_nrt_profile rc={n}")
                print(f"profile: {n} file(s) written to {output_dir}")

        return _hook

    def get_axon_ntff_profile_hook():
        if state["hook"] is None:
            try:
                state["hook"] = _build_hook()
            except Exception:
                state["hook"] = None
        return state["hook"]

    mod = types.ModuleType("antenv.axon_hooks")
    mod.get_axon_ntff_profile_hook = get_axon_ntff_profile_hook
    mod.set_axon_ntff_profile_hook = lambda h: state.update(hook=h)
    sys.modules["antenv.axon_hooks"] = mod


_install_ntff_hook_shim()

P = 128          # SBUF partitions
DIM = 1024       # model dim
KD = DIM // P    # 8 contraction tiles
ROWS_PER_CORE = 2048
SLABS_PER_CORE = 8
SLAB = 256       # rows per slab
N_CORES = 8
BF16 = mybir.dt.bfloat16
F32 = mybir.dt.float32

_CACHE = {}


def _build_graph():
    nc = bacc.Bacc("TRN2", target_bir_lowering=False, debug=False,
                   num_devices=N_CORES)

    xt_d = nc.dram_tensor("xt", [DIM, ROWS_PER_CORE], BF16, kind="ExternalInput")
    w_d = {
        name: nc.dram_tensor(name, [DIM, DIM], BF16, kind="ExternalInput")
        for name in ("wq", "wk", "wv", "wp")
    }
    bqc_d = nc.dram_tensor("bqc", [P, DIM], BF16, kind="ExternalInput")
    bkc_d = nc.dram_tensor("bkc", [P, DIM], BF16, kind="ExternalInput")
    bpc_d = nc.dram_tensor("bpc", [P, DIM], BF16, kind="ExternalInput")
    bvc_d = nc.dram_tensor("bvc", [P, KD], F32, kind="ExternalInput")
    ident_d = nc.dram_tensor("ident64", [64, 64], BF16, kind="ExternalInput")
    out_d = nc.dram_tensor("out", [ROWS_PER_CORE, DIM], F32, kind="ExternalOutput")

    with tile.TileContext(nc) as tc:
        with (
            tc.tile_pool(name="wpool", bufs=1) as wpool,
            tc.tile_pool(name="xpool", bufs=1) as xpool,
            tc.tile_pool(name="bias", bufs=1) as bias_pool,
            tc.tile_pool(name="qk", bufs=2) as qk_pool,
            tc.tile_pool(name="vt", bufs=2) as vt_pool,
            tc.tile_pool(name="ovt", bufs=2) as ovt_pool,
            tc.tile_pool(name="ysb", bufs=2) as y_pool,
            tc.tile_pool(name="soft", bufs=2) as soft_pool,
            tc.tile_pool(name="ps_proj", bufs=4, space="PSUM") as ps_proj_pool,
            tc.tile_pool(name="ps_s", bufs=1, space="PSUM") as ps_s_pool,
            tc.tile_pool(name="ps_ot", bufs=2, space="PSUM") as ps_ot_pool,
            tc.tile_pool(name="ps_wt", bufs=1, space="PSUM") as ps_wt_pool,
        ):
            # ---- resident tensors / DMA prologue ----------------------------
            # Emission order == per-ring enqueue order: load exactly what the
            # first matmuls need first (wv jt0 pieces + xt cols 0:512), then
            # the rest in order of first use.
            bq_bc = bias_pool.tile([P, DIM], BF16, tag="bqc")
            bk_bc = bias_pool.tile([P, DIM], BF16, tag="bkc")
            bp_bc = bias_pool.tile([P, DIM], BF16, tag="bpc")
            bv_col = bias_pool.tile([P, KD], F32, tag="bvc")
            ident = bias_pool.tile([64, 64], BF16, tag="ident")

            xt_sb = xpool.tile([P, KD, ROWS_PER_CORE], BF16, tag="xt")
            xt_src = xt_d[:].rearrange("(kd p) r -> p kd r", p=P)
            w_sb = {}
            for name in ("wq", "wk", "wv", "wp"):
                w_sb[name] = wpool.tile([P, KD, DIM], BF16, tag=f"w_{name}",
                                        name=f"w_{name}")
            w_srcs = {name: w_d[name][:].rearrange("(kd p) c -> p kd c", p=P)
                      for name in w_sb}

            # HAM pre-warm: ~12 junk matmuls on a memset tile keep the PE
            # busy from ~6us so the clock gate opens before real data lands
            warm = bias_pool.tile([P, 512], BF16, tag="warm")
            nc.vector.memset(warm[:], 0.0)
            ps_warm = ps_proj_pool.tile([P, 512], F32, tag="ps_proj")
            for i in range(12):
                nc.tensor.matmul(ps_warm[:], warm[:, 0:P], warm[:],
                                 start=(i == 0), stop=(i == 11))

            nc.sync.dma_start(bv_col[:], bvc_d[:])
            nc.sync.dma_start(ident[:], ident_d[:])
            # VT(0) inputs first, interleaved per kd so the first psum group
            # can start as soon as its own pieces land; pieces kept >=1KB
            # per partition row (descriptor-rate, not bandwidth, limits the
            # early load)
            for kd in range(KD):
                nc.sync.dma_start(xt_sb[:, kd, 0:512], xt_src[:, kd, 0:512])
                nc.sync.dma_start(w_sb["wv"][:, kd, :], w_srcs["wv"][:, kd, :])
            # then in order of first use
            for kd in range(KD):
                nc.sync.dma_start(w_sb["wq"][:, kd, 0:512],
                                  w_srcs["wq"][:, kd, 0:512])
            nc.sync.dma_start(bq_bc[:], bqc_d[:])
            for kd in range(KD):
                nc.sync.dma_start(w_sb["wk"][:, kd, 0:512],
                                  w_srcs["wk"][:, kd, 0:512])
            nc.sync.dma_start(bk_bc[:], bkc_d[:])
            for name in ("wq", "wk"):
                for kd in range(KD):
                    nc.sync.dma_start(w_sb[name][:, kd, 512:DIM],
                                      w_srcs[name][:, kd, 512:DIM])
            nc.sync.dma_start(bp_bc[:], bpc_d[:])
            for jc in range(2):
                for kd in range(KD):
                    nc.sync.dma_start(
                        w_sb["wp"][:, kd, jc * 512:(jc + 1) * 512],
                        w_srcs["wp"][:, kd, jc * 512:(jc + 1) * 512])
            # remaining xt (pairs 1-3)
            for half in range(1, 4):
                for kd in range(KD):
                    nc.sync.dma_start(
                        xt_sb[:, kd, half * 512:(half + 1) * 512],
                        xt_src[:, kd, half * 512:(half + 1) * 512])

            # ---- stage emitters ---------------------------------------------
            def emit_vt(pair):
                """V^T for both slabs of the pair (N=512 streams)."""
                vtt = vt_pool.tile([P, KD, 2 * SLAB], BF16, tag="vt")
                p0 = pair * 2 * SLAB
                for jt in range(KD):
                    ps = ps_proj_pool.tile([P, 512], F32, tag="ps_proj")
                    for kd in range(KD):
                        nc.tensor.matmul(
                            ps[:],
                            w_sb["wv"][:, kd, jt * P:(jt + 1) * P],
                            xt_sb[:, kd, p0: p0 + 512],
                            start=(kd == 0),
                            stop=(kd == KD - 1),
                        )
                    nc.scalar.activation(
                        vtt[:, jt, :], ps[:],
                        mybir.ActivationFunctionType.Identity,
                        bias=bv_col[:, jt: jt + 1])
                return vtt

            def proj_group(s, dst_t, wname, bias_bc, rt, jc):
                """One [128,512] psum group of a Q/K projection."""
                c0 = s * SLAB
                ps = ps_proj_pool.tile([P, 512], F32, tag="ps_proj")
                for kd in range(KD):
                    nc.tensor.matmul(
                        ps[:],
                        xt_sb[:, kd, c0 + rt * P: c0 + (rt + 1) * P],
                        w_sb[wname][:, kd, jc * 512:(jc + 1) * 512],
                        start=(kd == 0),
                        stop=(kd == KD - 1),
                    )
                nc.vector.tensor_add(
                    dst_t[:, rt, jc * 512:(jc + 1) * 512],
                    ps[:],
                    bias_bc[:, jc * 512:(jc + 1) * 512],
                )

            def emit_proj(s, dst_t, wname, bias_bc):
                """One of Q/K natural-layout projections for slab s."""
                for rt in range(2):
                    for jc in range(2):
                        proj_group(s, dst_t, wname, bias_bc, rt, jc)

            def emit_s_mms(q_nat, k_nat):
                """S = sum over (rt, t2) of Q_blk^T @ K_blk -> PSUM [64,64]."""
                ps_s = ps_s_pool.tile([64, 64], F32, tag="ps_s")
                n_acc = 0
                for rt in range(2):
                    for t2 in range(16):
                        nc.tensor.matmul(
                            ps_s[:],
                            q_nat[:, rt, t2 * 64:(t2 + 1) * 64],
                            k_nat[:, rt, t2 * 64:(t2 + 1) * 64],
                            start=(n_acc == 0),
                            stop=(n_acc == 31),
                        )
                        n_acc += 1
                return ps_s

            def emit_softmax(ps_s):
                """softmax over the free dim (DVE/ACT only)."""
                negmax = soft_pool.tile([64, 1], F32, tag="negmax")
                nc.vector.reduce_max(negmax[:], ps_s[:],
                                     axis=mybir.AxisListType.X, negate=True)
                p_sb = soft_pool.tile([64, 64], F32, tag="p_sb")
                rsum = soft_pool.tile([64, 1], F32, tag="rsum")
                nc.scalar.activation(p_sb[:], ps_s[:],
                                     mybir.ActivationFunctionType.Exp,
                                     bias=negmax[:], accum_out=rsum[:])
                rinv = soft_pool.tile([64, 1], F32, tag="rinv")
                nc.vector.reciprocal(rinv[:], rsum[:])
                w_soft = soft_pool.tile([64, 64], BF16, tag="w_soft")
                nc.vector.tensor_scalar_mul(w_soft[:], p_sb[:], rinv[:])
                return w_soft

            def emit_wt(w_soft):
                """P^T duplicated on both partition halves: two quadrant
                PE transposes into one [128,64] psum, one DVE evac."""
                ps_wt = ps_wt_pool.tile([P, 64], BF16, tag="ps_wt")
                nc.tensor.transpose(ps_wt[0:64, :], w_soft[:], ident[:])
                nc.tensor.transpose(ps_wt[64:128, :], w_soft[:], ident[:])
                wtd = soft_pool.tile([P, 64], BF16, tag="wtd")
                nc.vector.tensor_copy(wtd[:], ps_wt[:])
                return wtd

            def emit_ot(s, vtt, wtd):
                """O^T straight from vt, chunked by (g=jt, t3, parity): each
                stationary is a contiguous 64-col slice of vt; per (g,t3) the
                two parities run on disjoint array quadrants concurrently.
                The resulting psum partition scatter (c_dim = 16*(q%64) + 2g
                + (q>=64)) is absorbed into Wp's host-side row order."""
                ovt = ovt_pool.tile([P, KD, SLAB], BF16, tag="ovt")
                for g in range(KD):
                    ot_group(s, vtt, wtd, ovt, g)
                return ovt

            def ot_group(s, vtt, wtd, ovt, g):
                base = (s % 2) * SLAB
                pso = ps_ot_pool.tile([P, 4, 64], F32, tag="ps_ot")
                for t3 in range(4):
                    col = base + 64 * t3
                    nc.tensor.matmul(pso[0:64, t3, :],
                                     vtt[0:64, g, col:col + 64],
                                     wtd[0:64, :], start=True, stop=True)
                    nc.tensor.matmul(pso[64:128, t3, :],
                                     vtt[64:128, g, col:col + 64],
                                     wtd[64:128, :], start=True, stop=True)
                nc.vector.tensor_copy(
                    ovt[:, g, :].rearrange("p (d four) -> p d four", four=4),
                    pso[:].rearrange("p t3 d -> p d t3"),
                )

            def emit_y(s, ovt):
                """Y = OvT^T @ Wp_perm + bp (natural rows) -> DMA out."""
                y_sb = y_pool.tile([P, 2, DIM], F32, tag="y_sb")
                for rt in range(2):
                    for jc in range(2):
                        ps = ps_proj_pool.tile([P, 512], F32, tag="ps_proj")
                        for ct in range(KD):
                            nc.tensor.matmul(
                                ps[:],
                                ovt[:, ct, rt * P:(rt + 1) * P],
                                w_sb["wp"][:, ct, jc * 512:(jc + 1) * 512],
                                start=(ct == 0),
                                stop=(ct == KD - 1),
                            )
                        nc.vector.tensor_add(
                            y_sb[:, rt, jc * 512:(jc + 1) * 512],
                            ps[:],
                            bp_bc[:, jc * 512:(jc + 1) * 512],
                        )
                out_dst = out_d[s * SLAB:(s + 1) * SLAB, :] \
                    .rearrange("(rt p) c -> p rt c", p=P)
                for rt in range(2):
                    for jc in range(2):
                        nc.sync.dma_start(
                            out_dst[:, rt, jc * 512:(jc + 1) * 512],
                            y_sb[:, rt, jc * 512:(jc + 1) * 512])

            # ---- software pipeline ------------------------------------------
            vt_tiles = {0: emit_vt(0)}
            q_nat = qk_pool.tile([P, 2, DIM], BF16, tag="q_nat")
            k_nat = qk_pool.tile([P, 2, DIM], BF16, tag="k_nat")
            emit_proj(0, q_nat, "wq", bq_bc)
            emit_proj(0, k_nat, "wk", bk_bc)

            last = SLABS_PER_CORE - 1
            w_soft_next = None
            for s in range(SLABS_PER_CORE):
                pair = s // 2
                if w_soft_next is None:
                    ps_s = emit_s_mms(q_nat, k_nat)
                    w_soft = emit_softmax(ps_s)
                else:
                    w_soft = w_soft_next
                if s % 2 == 1 and pair + 1 < SLABS_PER_CORE // 2:
                    vt_tiles[pair + 1] = emit_vt(pair + 1)
                if s + 1 < SLABS_PER_CORE:
                    q_next = qk_pool.tile([P, 2, DIM], BF16, tag="q_nat")
                    k_next = qk_pool.tile([P, 2, DIM], BF16, tag="k_nat")
                    emit_proj(s + 1, q_next, "wq", bq_bc)
                else:
                    q_next = k_next = None
                wtd = emit_wt(w_soft)
                vtt = vt_tiles[pair]
                if k_next is not None:
                    # interleave O^T groups between K-projection groups so
                    # their LDWEIGHTS issue hides under the N=512 streams
                    ovt = ovt_pool.tile([P, KD, SLAB], BF16, tag="ovt")
                    groups = [(rt, jc) for rt in range(2) for jc in range(2)]
                    for i, (rt, jc) in enumerate(groups):
                        proj_group(s + 1, k_next, "wk", bk_bc, rt, jc)
                        ot_group(s, vtt, wtd, ovt, 2 * i)
                        ot_group(s, vtt, wtd, ovt, 2 * i + 1)
                else:
                    ovt = emit_ot(s, vtt, wtd)
                if s + 1 == last:
                    # hoist the last slab's S + softmax here so its serial
                    # DVE/ACT chain overlaps E(s)/F(s) instead of stalling
                    # the PE at the very end
                    ps_s7 = emit_s_mms(q_next, k_next)
                    w_soft_next = emit_softmax(ps_s7)
                emit_y(s, ovt)
                q_nat, k_nat = q_next, k_next

    nc.compile()
    return nc


def _prep_inputs(x, Wq, bq, Wk, bk, Wv, bv, Wp, bp):
    """Host-side shard prep. Returns in_maps list for 8 cores."""
    bf16 = ml_dtypes.bfloat16
    xf = np.ascontiguousarray(np.asarray(x, dtype=np.float32).reshape(-1, DIM))
    scale = np.float32(1.0 / np.sqrt(64.0))

    wq_b = np.ascontiguousarray((np.asarray(Wq) * scale).astype(bf16))
    wk_b = np.ascontiguousarray(np.asarray(Wk).astype(bf16))
    wv_b = np.ascontiguousarray(np.asarray(Wv).astype(bf16))
    # Absorb the O^T psum partition scatter into Wp's rows: contraction
    # chunk g, psum partition q holds O_view^T row c = 16*(q%64)+2g+(q>=64).
    q_idx = np.arange(P)
    src_rows = (16 * (q_idx % 64)[None, :] + 2 * np.arange(KD)[:, None]
                + (q_idx >= 64)[None, :])                 # [KD, P]
    wp_b = np.asarray(Wp).astype(bf16)[src_rows.reshape(-1), :]
    wp_b = np.ascontiguousarray(wp_b)

    bqc = np.ascontiguousarray(np.broadcast_to(
        (np.asarray(bq) * scale).astype(bf16), (P, DIM)))
    bkc = np.ascontiguousarray(np.broadcast_to(
        np.asarray(bk).astype(bf16), (P, DIM)))
    bpc = np.ascontiguousarray(np.broadcast_to(
        np.asarray(bp).astype(bf16), (P, DIM)))
    bvc = np.ascontiguousarray(
        np.asarray(bv, dtype=np.float32).reshape(KD, P).T)
    ident = np.eye(64, dtype=bf16)

    shared = {
        "wq": wq_b, "wk": wk_b, "wv": wv_b, "wp": wp_b,
        "bqc": bqc, "bkc": bkc, "bpc": bpc, "bvc": bvc,
        "ident64": ident,
    }
    in_maps = []
    for c in range(N_CORES):
        xs = xf[c * ROWS_PER_CORE:(c + 1) * ROWS_PER_CORE]  # [2048, 1024]
        xt = np.ascontiguousarray(xs.T.astype(bf16))        # [1024, 2048]
        in_maps.append({"xt": xt, **shared})
    return in_maps


def kernel(x, Wq, bq, Wk, bk, Wv, bv, Wp, bp):
    if "nc" not in _CACHE:
        _CACHE["nc"] = _build_graph()
    nc = _CACHE["nc"]

    in_maps = _prep_inputs(x, Wq, bq, Wk, bk, Wv, bv, Wp, bp)
    trace = bool(int(os.environ.get("ATHENA_TRACE", "0")))
    res = run_bass_kernel_spmd(nc, in_maps, core_ids=list(range(N_CORES)),
                               trace=trace)
    _CACHE["last_exec_time_ns"] = res.exec_time_ns

    out = np.concatenate([res.results[c]["out"] for c in range(N_CORES)], axis=0)
    return np.ascontiguousarray(out.reshape(np.asarray(x).shape)
                                .astype(np.float32))
